# revision 23
# baseline (speedup 1.0000x reference)
"""Trainium2 kernel v3: parity + diamond-truncated diamond-search motion.

Candidate-set cuts (device computes SAD cost sums only where the host walk
can read them):
- LDSP moves all have even (dy+dx) parity, so the LDSP walk only ever
  evaluates the even-parity checkerboard; the 4 odd-parity SDSP refinement
  costs are data-dependent and computed exactly in fp32 on host (making the
  SDSP decision exact).
- Walks rarely stray far: candidates are restricted to the |dy|+|dx| <= 4
  diamond (25 of 289 points).  Blocks whose walk candidates ever leave the
  diamond (~31% on this input; flagged at the first step whose candidates
  exit, i.e. before the truncated walk can diverge) are recomputed exactly
  on host, as are blocks with any fp16 LDSP argmin margin < 2*TAU.
- The 17px output crop makes border blocks irrelevant: 60x60 interior
  blocks, 480x480 pixels, every remaining candidate in-bounds (no padding
  or validity masking on device).

Device engine split per 120-row chunk x 17 dy (see SUB/ABS/CPY_MODES):
- subs d = P - I_dy,dx: DVE (fp16 2x) for most dys, GPSIMD TensorTensor for
  5 mid-size dys (the only elementwise op walrus accepts on Pool).
- |d|: ACT activation Abs / DVE uint16 &0x7fff (4x mode, exact fp16 abs).
- one fp16 pairwise fold (8 -> 4 col-lanes) on DVE, then PE accumulates the
  remaining 4x8-row reduction into f32 PSUM via 4 stride-4 matmul lanes
  against a 0/1 row-selector (fewer PE instructions; matmul issue costs
  ~350ns each on HW, which made 8 lanes PE-bound).
- PSUM->SBUF copies on ACT/DVE (GPSIMD cannot read PSUM); DMA out as f32.
"""
import numpy as np
from contextlib import ExitStack

import concourse.bass as bass
import concourse.bacc as bacc
import concourse.mybir as mybir
import concourse.tile as tile
from concourse.alu_op_type import AluOpType
from concourse.bass_utils import run_bass_kernel_spmd

MB = 8
P = 8
CROP = 17
LARGE_SUM = np.float32(65537.0 * 64)
MAX_STEPS = 16
LDSP = np.array([[0, -2], [-1, -1], [1, -1], [-2, 0], [0, 0], [2, 0],
                 [-1, 1], [1, 1], [0, 2]], dtype=np.int32)
SDSP = np.array([[0, -1], [-1, 0], [0, 0], [1, 0], [0, 1]], dtype=np.int32)

B, T, H, W = 4, 16, 512, 512
NBR = 60                 # interior block rows (2..61 of the original 64)
NBC = 60
TT = T - 2
NPAIR = B * TT           # 56 motion fields consumed
CHUNKS = 4               # 120-row chunks
BI = 15                  # block rows per chunk
NUNIT = NPAIR * CHUNKS   # 224
NCORES = 8
UPC = NUNIT // NCORES    # 28

# R L1-diamond of even-(dy+dx) candidates: per dy, |dx| <= R-|dy| with
# dx = dy (mod 2).  The LDSP walk is repaired exactly on host for blocks
# whose walk candidates ever leave the diamond (R=4: ~31% on this input;
# R=6: ~8%; R=8: ~1.8% — R=4 won on HW, fixed costs dominate below it).
RDIAM = 4
NDX = [max(0, RDIAM + 1 - abs(d - 8)) for d in range(17)]
DXI0 = [abs(d - 8) + (8 - RDIAM) for d in range(17)]  # first dxi per dy
DY0 = 8 - RDIAM          # first dyi with candidates
NDYS = 2 * RDIAM + 1
VOLW = 540

# sub engine per dyi: 'D' = DVE (fp16 2x), 'G' = GPSIMD (TensorTensor; the
# only elementwise op walrus accepts on Pool, which also cannot read PSUM)
SUB_MODES = "DDDDDGDGDGDDDDDDD"
# abs engine per dyi: 'A' = ACT activation Abs, 'V' = DVE uint16 &0x7fff (4x)
ABS_MODES = "VVVVVVAAAAAVVVVVV"
# psum->sbuf copy engine per dyi: 'A'/'V'
CPY_MODES = "AAAAAAAAAAAAAVAVA"
# fp16-pipeline error bound on cost sums; blocks with any LDSP argmin margin
# < 2*TAU along the walk are recomputed exactly on host.
TAU = np.float32(0.0833)

_CACHED_NC = None


def _build_nc(nproc=UPC, static=True, repeat=1, abs_modes=ABS_MODES,
              cpy_modes=CPY_MODES, bufs=12, stages="safm", psum_bufs=8,
              sub_modes=SUB_MODES, ubufs=2, vbufs=8, fbufs=10, lanes=2):
    """stages: 's' sub, 'a' abs, 'f' fold, 'm' matmul+copy+dma."""
    nc = bacc.Bacc()
    f16 = mybir.dt.float16
    f32 = mybir.dt.float32
    xP = nc.dram_tensor("xP", [UPC * 120, 480], f16, kind="ExternalInput")
    xI = nc.dram_tensor("xI", [UPC * 136, 496], f16, kind="ExternalInput")
    sel = nc.dram_tensor("sel", [120, BI], f16, kind="ExternalInput")
    vol = nc.dram_tensor("vol", [UPC * 17 * BI, VOLW], f32,
                         kind="ExternalOutput")

    Abs = mybir.ActivationFunctionType.Abs

    with tile.TileContext(nc) as tc, ExitStack() as ctx, \
            nc.allow_low_precision(reason="fp16 SAD partials; host repairs "
                                          "low-margin argmins"):
        cpool = ctx.enter_context(tc.tile_pool(name="cpool", bufs=1))
        upool = ctx.enter_context(tc.tile_pool(name="upool", bufs=ubufs))
        wpool = ctx.enter_context(tc.tile_pool(name="wpool", bufs=bufs))
        fpool = ctx.enter_context(tc.tile_pool(name="fpool", bufs=fbufs))
        vpool = ctx.enter_context(tc.tile_pool(name="vpool", bufs=vbufs))
        psum = ctx.enter_context(tc.tile_pool(name="psum", bufs=psum_bufs,
                                              space="PSUM"))

        sel_t = cpool.tile([120, BI], f16, tag="sel")
        nc.sync.dma_start(sel_t[:, :], sel[:, :])

        # emission order: alternate GPS-sub and DVE-sub dys so both sub
        # engines get work immediately at each unit boundary.
        gpss = [d for d in range(17) if sub_modes[d] == "G" and NDX[d] > 0]
        dves = [d for d in range(17) if sub_modes[d] == "D" and NDX[d] > 0]
        order = []
        for i in range(max(len(gpss), len(dves))):
            if i < len(dves):
                order.append(dves[i])
            if i < len(gpss):
                order.append(gpss[i])

        def unit_body(u):
            p_t = upool.tile([120, 480], f16, tag="p")
            i17 = upool.tile([120, NDYS, 496], f16, tag="i17")
            nc.sync.dma_start(p_t[:, :], xP[bass.ts(u, 120), :])
            src = xI[bass.ts(u, 136), :]
            rep = bass.AP(src.tensor, offset=src.offset + DY0 * 496,
                          ap=[[496, 120], [1, NDYS * 496]])
            i17v = i17[:, :, :]
            nc.sync.dma_start(
                bass.AP(i17v.tensor, offset=i17v.offset,
                        ap=[i17v.ap[0], [1, NDYS * 496]]), rep)

            # Software-pipelined emission: each engine's hardware queue is
            # in-order, so emitting a dy's whole chain contiguously makes
            # consumers (fold on DVE, copy on ACT) head-of-line block work
            # that is already ready (the next dy's sub/abs).  Stage-shift
            # the chains instead: produce(k) | reduce(k-1) | out(k-2).
            state = {}

            def produce(dyi):
                ndx = NDX[dyi]
                d_t = wpool.tile([120, 9, 480], f16, tag="d")
                dv = d_t[:, :, :]
                dout = bass.AP(dv.tensor, offset=dv.offset,
                               ap=[dv.ap[0], [480, ndx], [1, 480]])
                pv = p_t[:, :]
                in0 = bass.AP(pv.tensor, offset=pv.offset,
                              ap=[pv.ap[0], [0, ndx], [1, 480]])
                iv = i17[:, :, :]
                in1 = bass.AP(iv.tensor,
                              offset=iv.offset + (dyi - DY0) * 496
                                     + DXI0[dyi],
                              ap=[iv.ap[0], [2, ndx], [1, 480]])
                if "s" in stages:
                    seng = nc.gpsimd if sub_modes[dyi] == "G" else nc.vector
                    seng.tensor_sub(dout, in0, in1)
                if "a" in stages and "s" in stages:
                    if abs_modes[dyi] == "A":
                        flat = bass.AP(dv.tensor, offset=dv.offset,
                                       ap=[dv.ap[0], [1, ndx * 480]])
                        nc.scalar.activation(flat, flat, Abs)
                    else:
                        du = bass.AP(dv.tensor, offset=dv.offset,
                                     ap=[dv.ap[0], [1, ndx * 480]]
                                     ).bitcast(mybir.dt.uint16)
                        nc.vector.tensor_scalar(du, du, 0x7FFF, None,
                                                AluOpType.bitwise_and)
                state[dyi] = dv

            def reduce(dyi):
                ndx = NDX[dyi]
                G = ndx * NBC
                dv = state[dyi]
                f_t = fpool.tile([120, 9, 360 if lanes == 2 else 240], f16,
                                 tag="f")
                fv = f_t[:, :, :]
                if "f" in stages:
                    s0 = bass.AP(dv.tensor, offset=dv.offset,
                                 ap=[dv.ap[0], [8, G], [1, 4]])
                    s1 = bass.AP(dv.tensor, offset=dv.offset + 4,
                                 ap=[dv.ap[0], [8, G], [1, 4]])
                    do = bass.AP(fv.tensor, offset=fv.offset,
                                 ap=[fv.ap[0], [4, G], [1, 4]])
                    nc.vector.tensor_add(do, s0, s1)
                    if lanes == 2:
                        t0 = bass.AP(fv.tensor, offset=fv.offset,
                                     ap=[fv.ap[0], [4, G], [1, 2]])
                        t1 = bass.AP(fv.tensor, offset=fv.offset + 2,
                                     ap=[fv.ap[0], [4, G], [1, 2]])
                        to = bass.AP(fv.tensor, offset=fv.offset + 9 * 240,
                                     ap=[fv.ap[0], [2, G], [1, 2]])
                        nc.vector.tensor_add(to, t0, t1)
                if "m" not in stages:
                    state[dyi] = None
                    return
                fbase = fv.offset if lanes == 4 else fv.offset + 9 * 240
                ps = psum.tile([BI, 512 if G <= 512 else 1024],
                               mybir.dt.float32, tag="ps")
                regions = [(0, G)] if G <= 512 else [(0, 512), (512, G)]
                for g0, g1 in regions:
                    for v in range(lanes):
                        rhs = bass.AP(fv.tensor,
                                      offset=fbase + lanes * g0 + v,
                                      ap=[fv.ap[0], [lanes, g1 - g0]])
                        nc.tensor.matmul(ps[:, g0:g1], sel_t[:, :], rhs,
                                         start=(v == 0),
                                         stop=(v == lanes - 1))
                state[dyi] = ps

            def out(dyi):
                if "m" not in stages:
                    return
                ndx = NDX[dyi]
                G = ndx * NBC
                ps = state[dyi]
                vs = vpool.tile([BI, VOLW], mybir.dt.float32, tag="vs")
                cm = cpy_modes[dyi]
                if cm == "A":
                    nc.scalar.copy(vs[:, :G], ps[:, :G])
                else:  # 'V' (GPSIMD cannot access PSUM)
                    nc.vector.tensor_copy(vs[:, :G], ps[:, :G])
                nc.sync.dma_start(vol[bass.ds((u * 17 + dyi) * BI, BI), :G],
                                  vs[:, :G])

            n = len(order)
            for k in range(n + 2):
                if k < n:
                    produce(order[k])
                if 1 <= k < n + 1:
                    reduce(order[k - 1])
                if k >= 2:
                    out(order[k - 2])

        if static:
            if repeat > 1:
                with tc.For_i(0, repeat, 1) as _r:
                    for u in range(nproc):
                        unit_body(u)
            else:
                for u in range(nproc):
                    unit_body(u)
        else:
            with tc.For_i(0, nproc, 1) as u:
                unit_body(u)

    nc.compile()
    return nc


def _get_nc():
    global _CACHED_NC
    if _CACHED_NC is None:
        _CACHED_NC = _build_nc(UPC, static=True)
    return _CACHED_NC


def _unit_list():
    return [(b, t, c) for b in range(B) for t in range(TT)
            for c in range(CHUNKS)]


def _pack_inputs(vids):
    """Per-core xP/xI buffers (fp16).  vids: (B, T, 512, 512) f32."""
    v16 = vids.astype(np.float16)
    units = _unit_list()
    sel = (np.arange(120)[:, None] // 8 == np.arange(BI)[None, :])
    sel = np.ascontiguousarray(sel, np.float16)
    in_maps = []
    assign = []
    for k in range(NCORES):
        mine = units[k::NCORES]
        assign.append(mine)
        xP = np.empty((UPC, 120, 480), np.float16)
        xI = np.empty((UPC, 136, 496), np.float16)
        for i, (b, t, c) in enumerate(mine):
            r0 = 16 + 120 * c
            xP[i] = v16[b, t + 1, r0:r0 + 120, 16:496]
            xI[i] = v16[b, t, r0 - 8:r0 + 128, 8:504]
        in_maps.append({"xP": xP.reshape(UPC * 120, 480),
                        "xI": xI.reshape(UPC * 136, 496),
                        "sel": sel})
    return in_maps, assign


def _assemble_vols(results, assign):
    """-> vol (NPAIR, 60, 60, 17, 17) f32; odd-parity entries = LARGE_SUM."""
    vol = np.full((NPAIR, NBR, NBC, 17, 17), LARGE_SUM, np.float32)
    for k in range(NCORES):
        out = np.asarray(results[k]["vol"]).reshape(UPC, 17, BI, VOLW)
        for i, (b, t, c) in enumerate(assign[k]):
            pair = b * TT + t
            for dyi in range(17):
                ndx = NDX[dyi]
                if ndx == 0:
                    continue
                blk = out[i, dyi, :, :ndx * NBC].reshape(BI, ndx, NBC)
                vol[pair, BI * c:BI * (c + 1), :, dyi,
                    DXI0[dyi]:DXI0[dyi] + 2 * ndx:2] = blk.transpose(0, 2, 1)
    return vol


def _valid(ny, nx):
    return (np.abs(ny) <= P) & (np.abs(nx) <= P)


def _walk(vol):
    """LDSP diamond walk on the truncated parity cost volume.  Returns
    (cy, cx, margin, oob): end positions, the minimum argmin margin along
    the walk (incl. the c0==0 decision), and an out-of-diamond flag for
    blocks whose candidates ever left the R=8 diamond (their walk may have
    read LARGE placeholders -> host recomputes them exactly)."""
    lead = vol.shape[:-2]
    N = int(np.prod(lead))
    v = vol.reshape(N, 17, 17)
    cy = np.zeros(N, np.int32)
    cx = np.zeros(N, np.int32)
    margin = np.abs(v[:, 8, 8]).astype(np.float32)
    done = v[:, 8, 8] == 0.0
    oob = np.zeros(N, bool)
    rows = np.arange(N)
    for _ in range(MAX_STEPS):
        ny = cy[:, None] + LDSP[None, :, 1]
        nx = cx[:, None] + LDSP[None, :, 0]
        ok = _valid(ny, nx)
        c = v[rows[:, None], np.clip(ny, -8, 8) + 8, np.clip(nx, -8, 8) + 8]
        c = np.where(ok, c, LARGE_SUM)
        pt = np.argmin(c, axis=1)
        move = ~done
        oob |= move & (np.abs(cy) + np.abs(cx) + 2 > RDIAM)
        s = np.partition(c, 1, axis=1)
        margin = np.where(move, np.minimum(margin, s[:, 1] - s[:, 0]), margin)
        cy = np.where(move, cy + LDSP[pt, 1], cy)
        cx = np.where(move, cx + LDSP[pt, 0], cx)
        done |= pt == 4
        if done.all():
            break
    return cy, cx, margin, oob


def _sdsp_exact(vids, cy, cx):
    """Exact fp32 SDSP refinement for every block.  cy/cx: (N,) int32 LDSP
    end positions, N = NPAIR*3600.  Returns refined (cy, cx)."""
    N = cy.shape[0]
    pairs = np.arange(N) // (NBR * NBC)
    bis = (np.arange(N) // NBC) % NBR
    bjs = np.arange(N) % NBC
    bb = pairs // TT
    tt = pairs % TT
    u8 = np.arange(MB)
    costs = np.empty((N, 5), np.float32)
    py = (bis + 2) * MB
    px = (bjs + 2) * MB
    blkP = vids[bb[:, None, None], tt[:, None, None] + 1,
                py[:, None, None] + u8[None, :, None],
                px[:, None, None] + u8[None, None, :]]
    for j in range(5):
        dy2 = cy + SDSP[j, 1]
        dx2 = cx + SDSP[j, 0]
        ok = _valid(dy2, dx2)
        ry = py + np.clip(dy2, -P, P)
        rx = px + np.clip(dx2, -P, P)
        win = vids[bb[:, None, None], tt[:, None, None],
                   ry[:, None, None] + u8[None, :, None],
                   rx[:, None, None] + u8[None, None, :]]
        cst = np.abs(blkP - win).sum((-1, -2), dtype=np.float32)
        costs[:, j] = np.where(ok, cst, LARGE_SUM)
    spt = np.argmin(costs, axis=1)
    return cy + SDSP[spt, 1], cx + SDSP[spt, 0]


def _repair(vids, cy, cx, margin, oob):
    """Recompute the full walk exactly (fp32) for blocks whose LDSP margin is
    below 2*TAU or whose walk left the R-diamond.  Active-set compaction:
    finished blocks drop out of the per-step cost gathers."""
    flags = (margin < 2 * TAU) | oob
    idx = np.nonzero(flags)[0]
    if idx.size == 0:
        return cy, cx, 0
    pairs = idx // (NBR * NBC)
    bis = ((idx // NBC) % NBR).astype(np.int64)
    bjs = (idx % NBC).astype(np.int64)
    bb = (pairs // TT).astype(np.int64)
    tt = (pairs % TT).astype(np.int64)
    F = len(idx)
    u8 = np.arange(MB)
    py = (bis + 2) * MB
    px = (bjs + 2) * MB
    blkP = vids[bb[:, None, None], tt[:, None, None] + 1,
                py[:, None, None] + u8[None, :, None],
                px[:, None, None] + u8[None, None, :]]

    def costs(sub, ry0, rx0, dsp):
        ny = ry0[:, None] + dsp[None, :, 1]
        nx = rx0[:, None] + dsp[None, :, 0]
        ok = _valid(ny, nx)
        ry = py[sub][:, None] + np.clip(ny, -P, P)
        rx = px[sub][:, None] + np.clip(nx, -P, P)
        win = vids[bb[sub][:, None, None, None], tt[sub][:, None, None, None],
                   ry[:, :, None, None] + u8[None, None, :, None],
                   rx[:, :, None, None] + u8[None, None, None, :]]
        c = np.abs(blkP[sub][:, None] - win).sum((-1, -2), dtype=np.float32)
        return np.where(ok, c, LARGE_SUM)

    ry = np.zeros(F, np.int32)
    rx = np.zeros(F, np.int32)
    allf = np.arange(F)
    c0 = costs(allf, ry, rx, np.array([[0, 0]], np.int32))[:, 0]
    active = np.nonzero(c0 != 0.0)[0]
    for _ in range(MAX_STEPS):
        if active.size == 0:
            break
        c = costs(active, ry[active], rx[active], LDSP)
        pt = np.argmin(c, axis=1)
        ry[active] += LDSP[pt, 1]
        rx[active] += LDSP[pt, 0]
        active = active[pt != 4]
    c = costs(allf, ry, rx, SDSP)
    spt = np.argmin(c, axis=1)
    ry = ry + SDSP[spt, 1]
    rx = rx + SDSP[spt, 0]
    cy = cy.copy()
    cx = cx.copy()
    cy[idx] = ry
    cx[idx] = rx
    return cy, cx, F


def _compensate(vids, cy, cx):
    """pred frames from interior motion; border blocks are cropped anyway."""
    m = np.zeros((B, TT, 64, 64, 2), np.int32)
    m[:, :, 2:62, 2:62, 0] = cy.reshape(B, TT, NBR, NBC)
    m[:, :, 2:62, 2:62, 1] = cx.reshape(B, TT, NBR, NBC)
    b_idx = np.arange(B)[:, None, None, None]
    t_idx = np.arange(TT)[None, :, None, None]
    ys = np.arange(64)[None, None, :, None] * MB + m[:, :, :, :, 0]
    xs = np.arange(64)[None, None, None, :] * MB + m[:, :, :, :, 1]
    rows = ys[..., None, None] + np.arange(MB)[None, None, None, None, :, None]
    cols = xs[..., None, None] + np.arange(MB)[None, None, None, None, None, :]
    src = vids[:, 1:T - 1]
    blocks = src[b_idx[..., None, None], t_idx[..., None, None], rows, cols]
    return blocks.transpose(0, 1, 2, 4, 3, 5).reshape(B, TT, H, W)


def kernel(x):
    x = np.ascontiguousarray(np.asarray(x), dtype=np.float32)
    vids = x[:, 0]
    in_maps, assign = _pack_inputs(vids)
    nc = _get_nc()
    res = run_bass_kernel_spmd(nc, in_maps, core_ids=list(range(NCORES)))
    vol = _assemble_vols(res.results, assign)
    cy, cx, margin, oob = _walk(vol)
    cy, cx = _sdsp_exact(vids, cy, cx)
    cy, cx, nrep = _repair(vids, cy, cx, margin, oob)
    pred = _compensate(vids, cy, cx)[:, :, CROP:-CROP, CROP:-CROP]
    target = vids[:, 2:, CROP:-CROP, CROP:-CROP]
    return target[:, None].copy(), pred[:, None].copy()


if __name__ == "__main__":
    x = np.load("/tmp/x_input.npy")
    t, p = kernel(x)
    et = np.load("/tmp/exp_target.npy")
    ep = np.load("/tmp/exp_pred.npy")
    print("target equal:", np.array_equal(t, et))
    print("pred equal:", np.array_equal(p, ep))
    d = p - ep
    print("n diff:", int((d != 0).sum()), "rel:",
          float(np.linalg.norm(d.ravel()) / np.linalg.norm(ep.ravel())))
